# revision 3
# baseline (speedup 1.0000x reference)
"""Trainium2 Bass kernel v3 for 16-head causal self-attention (KaplanAttention).

Sharding (8 cores): core c handles batch b = c // 4 and head group g = c % 4
(heads 4g..4g+3); host sums the 4 partial output projections per batch.

v3 = v2 (s-major streaming attention, flipped AV for per-partition softmax
normalization, deferred PE transposes) with the emission re-ordered so the
PE stream never goes sparse (sparse PE drops the HAM clock gate to 1.2 GHz):
  - input DMAs interleaved (weights early) so the first projection matmul
    issues ~3us in instead of ~20us;
  - V and the hp=1 Q/K projections are woven into the hp=0 attention stream;
  - the final output projection is woven into the hp=1 attention stream;
  - tail PSUM->SBUF copies alternate DVE/ACT.
"""

import numpy as np

from concourse import bass_utils, mybir, tile
from concourse import bacc

S = 2048
D = 1024
HPC = 4        # heads per core
DK = 64
DC = HPC * DK  # 256 d-columns per core
NCORES = 8
EC = D // 128  # 8 e-chunks
NJT = S // 128  # 16 j-tiles
NST = S // 512  # 4 s-windows of 512

FP16 = mybir.dt.float16
FP32 = mybir.dt.float32


def _build():
    nc = bacc.Bacc("TRN2", target_bir_lowering=False, debug=False)

    xT_d = nc.dram_tensor("xT", [D, S], FP16, kind="ExternalInput")
    wq_d = nc.dram_tensor("wqT", [D, DC], FP16, kind="ExternalInput")
    wk_d = nc.dram_tensor("wkT", [D, DC], FP16, kind="ExternalInput")
    wv_d = nc.dram_tensor("wvT", [D, DC], FP16, kind="ExternalInput")
    wo_d = nc.dram_tensor("woT", [DC, D], FP16, kind="ExternalInput")
    mask_d = nc.dram_tensor("mask", [128, 2 * 128], FP16, kind="ExternalInput")
    id_d = nc.dram_tensor("ident", [128, 128], FP16, kind="ExternalInput")
    out_d = nc.dram_tensor("out", [S, D], FP32, kind="ExternalOutput")

    with tile.TileContext(nc) as tc:
        with (
            tc.tile_pool(name="const", bufs=1) as const,
            tc.tile_pool(name="work", bufs=1) as work,
            tc.tile_pool(name="upool", bufs=2) as upool,
            tc.tile_pool(name="ospool", bufs=10) as ospool,
            tc.tile_pool(name="zpool", bufs=8) as zpool,
            tc.tile_pool(name="obpool", bufs=3) as obpool,
            tc.tile_pool(name="psBig", bufs=2, space="PSUM") as psBig,
            tc.tile_pool(name="psPo", bufs=3, space="PSUM") as psPo,
            tc.tile_pool(name="psPt", bufs=1, space="PSUM") as psPt,
        ):
            # ---- load inputs (weights early so compute starts right away) ----
            xT = const.tile([128, EC, S], FP16)
            wq = const.tile([128, EC, DC], FP16)
            wk = const.tile([128, EC, DC], FP16)
            wv = const.tile([128, EC, DC], FP16)
            wo = const.tile([128, 2, D], FP16)
            maskD = const.tile([128, 2, 128], FP16)
            ident = const.tile([128, 128], FP16)

            def load_x(c):
                nc.sync.dma_start(
                    out=xT[:, c, :], in_=xT_d[128 * c : 128 * (c + 1), :]
                )

            nc.sync.dma_start(out=wq, in_=wq_d.rearrange("(c p) d -> p c d", p=128))
            load_x(0)
            nc.sync.dma_start(out=wk, in_=wk_d.rearrange("(c p) d -> p c d", p=128))
            load_x(1)
            nc.sync.dma_start(out=wv, in_=wv_d.rearrange("(c p) d -> p c d", p=128))
            load_x(2)
            nc.sync.dma_start(out=wo, in_=wo_d.rearrange("(c p) d -> p c d", p=128))
            load_x(3)
            nc.sync.dma_start(
                out=maskD, in_=mask_d.rearrange("p (t c) -> p t c", t=2)
            )
            nc.sync.dma_start(out=ident, in_=id_d[:, :])
            for c in range(4, EC):
                load_x(c)

            QT = work.tile([128, 2, S], FP16)
            KT = work.tile([128, 2, S], FP16)
            V = work.tile([128, NJT, HPC, 65], FP16)
            nc.vector.memset(V[:, :, :, 64:65], 1.0)
            outTn = work.tile([128, 2, S], FP16)  # [d-of-pair, hp, s], normalized

            def proj_qk(w_t, dst, hp, st):
                ps = psBig.tile([128, 2, 512], FP32, tag="big")
                for c in range(EC):
                    nc.tensor.matmul(
                        ps[:, 0, :],
                        w_t[:, c, 128 * hp : 128 * (hp + 1)],
                        xT[:, c, 512 * st : 512 * (st + 1)],
                        start=(c == 0),
                        stop=(c == EC - 1),
                    )
                nc.vector.tensor_copy(
                    out=dst[:, hp, 512 * st : 512 * (st + 1)], in_=ps[:, 0, :]
                )

            def proj_v(jt):
                ps = psBig.tile([128, 2, 512], FP32, tag="big")
                psd = ps[:, 0, 0:DC]
                for c in range(EC):
                    nc.tensor.matmul(
                        psd,
                        xT[:, c, 128 * jt : 128 * (jt + 1)],
                        wv[:, c, :],
                        start=(c == 0),
                        stop=(c == EC - 1),
                    )
                nc.vector.tensor_copy(
                    out=V[:, jt, :, 0:64],
                    in_=psd.rearrange("p (h d) -> p h d", h=HPC),
                )

            # deferred PE transposes: (hp, sb, os_tile)
            pending_t = []

            def emit_transpose():
                hp, sb, os_t = pending_t.pop(0)
                pt = psPt.tile([128, 128], FP16, tag="pt")
                nc.tensor.transpose(pt, os_t, ident)
                nc.vector.tensor_copy(
                    out=outTn[:, hp, 128 * sb : 128 * (sb + 1)], in_=pt
                )

            def attn_st(hp, st):
                Ut = upool.tile([128, 2, NJT, 512], FP16, tag="U")
                for jt in range(4 * st + 4):
                    off = max(0, 128 * jt - 512 * st)
                    n = 512 - off
                    ps = psBig.tile([128, 2, 512], FP32, tag="big")
                    for hi in range(2):
                        ho = 64 * hi
                        nc.tensor.matmul(
                            ps[:, hi, 0:n],
                            KT[ho : ho + 64, hp, 128 * jt : 128 * (jt + 1)],
                            QT[ho : ho + 64, hp, 512 * st + off : 512 * (st + 1)],
                            start=True,
                            stop=True,
                        )
                    nc.scalar.activation(
                        out=Ut[:, :, jt, off : off + n],
                        in_=ps[:, :, 0:n],
                        func=mybir.ActivationFunctionType.Exp,
                        scale=0.125,
                    )
                    if jt >= 4 * st:  # diagonal 128-block: causal mask
                        nc.vector.tensor_mul(
                            Ut[:, :, jt, off : off + 128],
                            Ut[:, :, jt, off : off + 128],
                            maskD,
                        )
                for sbl in range(4):
                    sb = 4 * st + sbl
                    po = psPo.tile([128, 2, 65], FP32, tag="po")
                    for hi in range(2):
                        for k in range(sb + 1):
                            nc.tensor.matmul(
                                po[:, hi, :],
                                Ut[:, hi, k, 128 * sbl : 128 * (sbl + 1)],
                                V[:, k, 2 * hp + hi, :],
                                start=(k == 0),
                                stop=(k == sb),
                            )
                    zr = zpool.tile([128, 2, 1], FP32, tag="zr")
                    nc.vector.reciprocal(out=zr, in_=po[:, :, 64:65])
                    os_t = ospool.tile([128, 2, DK], FP16, tag="os")
                    for hi in range(2):
                        nc.vector.tensor_scalar_mul(
                            os_t[:, hi, :], po[:, hi, 0:64], zr[:, hi, :]
                        )
                    pending_t.append((hp, sb, os_t))
                    if len(pending_t) > 4:
                        emit_transpose()

            def emit_final(sb):
                # transposes this final depends on must be emitted first
                while pending_t and (
                    pending_t[0][0] == 0 or pending_t[0][1] <= sb
                ):
                    emit_transpose()
                psf = psBig.tile([128, 2, 512], FP32, tag="big")
                for mt in range(2):
                    for hp in range(2):
                        nc.tensor.matmul(
                            psf[:, mt, :],
                            outTn[:, hp, 128 * sb : 128 * (sb + 1)],
                            wo[:, hp, 512 * mt : 512 * (mt + 1)],
                            start=(hp == 0),
                            stop=(hp == 1),
                        )
                ob = obpool.tile([128, 2, 512], FP32, tag="ob")
                if sb % 2 == 0:
                    nc.vector.tensor_copy(out=ob, in_=psf)
                else:
                    nc.scalar.copy(out=ob, in_=psf)
                nc.sync.dma_start(
                    out=out_d[128 * sb : 128 * (sb + 1), :].rearrange(
                        "p (t c) -> p t c", t=2
                    ),
                    in_=ob,
                )

            # ---- phase A: Q/K projections for hp=0 ----
            for st in range(NST):
                proj_qk(wq, QT, 0, st)
                proj_qk(wk, KT, 0, st)

            # ---- phase B: attention hp=0, V + hp=1 projections woven in ----
            for st in range(NST):
                for jt in range(4 * st, 4 * st + 4):
                    proj_v(jt)
                proj_qk(wq, QT, 1, st)
                proj_qk(wk, KT, 1, st)
                attn_st(0, st)

            # ---- phase C: attention hp=1, final projection woven in ----
            for st in range(NST):
                attn_st(1, st)
                if st >= 1:
                    for sb in range(4 * (st - 1), 4 * st):
                        emit_final(sb)
            for sb in range(12, 16):
                emit_final(sb)

    nc.compile()
    return nc


_NC = None


def _prep_in_maps(x, W_q, W_k, W_v, W_o):
    x = np.asarray(x, dtype=np.float32)
    W_q = np.asarray(W_q, dtype=np.float32)
    W_k = np.asarray(W_k, dtype=np.float32)
    W_v = np.asarray(W_v, dtype=np.float32)
    W_o = np.asarray(W_o, dtype=np.float32)
    mask01 = np.triu(np.ones((128, 128), dtype=np.float16))
    mask2 = np.concatenate([mask01, mask01], axis=1)
    ident = np.eye(128, dtype=np.float16)
    in_maps = []
    for c in range(NCORES):
        b, g = divmod(c, 4)
        cols = slice(DC * g, DC * (g + 1))
        in_maps.append(
            {
                "xT": np.ascontiguousarray(x[b].T).astype(np.float16),
                "wqT": np.ascontiguousarray(W_q[cols, :].T).astype(np.float16),
                "wkT": np.ascontiguousarray(W_k[cols, :].T).astype(np.float16),
                "wvT": np.ascontiguousarray(W_v[cols, :].T).astype(np.float16),
                "woT": np.ascontiguousarray(W_o[:, cols].T).astype(np.float16),
                "mask": mask2,
                "ident": ident,
            }
        )
    return in_maps


def _run(x, W_q, W_k, W_v, W_o, **spmd_kwargs):
    global _NC
    if _NC is None:
        _NC = _build()
    in_maps = _prep_in_maps(x, W_q, W_k, W_v, W_o)
    res = bass_utils.run_bass_kernel_spmd(
        _NC, in_maps, core_ids=list(range(NCORES)), **spmd_kwargs
    )
    parts = [res.results[c]["out"] for c in range(NCORES)]
    out = np.empty((2, S, D), dtype=np.float32)
    for b in range(2):
        out[b] = parts[4 * b] + parts[4 * b + 1] + parts[4 * b + 2] + parts[4 * b + 3]
    return out, res


def kernel(x, W_q, W_k, W_v, W_o):
    out, _ = _run(x, W_q, W_k, W_v, W_o)
    return out
